# revision 7
# baseline (speedup 1.0000x reference)
"""AttentionBlock kernel for Trainium2, data-parallel over batch on 8 cores.

Problem (hardcoded): x [16, 512, 32, 32] f32, GroupNorm(32 groups) ->
qkv 1x1 conv (8 heads, head dim 64) -> softmax attention -> proj 1x1
conv -> residual.  Each core handles 2 batches; no collectives.

Structure (v2, rebuilt from trace analysis of the previous version):
  - j-outer attention: q/k for all 4 head-pairs are computed up front
    (k for the whole batch, q per t-half), so the attention inner loop
    is a flat software pipeline of 32 steps per (batch, j):
      w-pair matmul (row-packed two heads at K=64) -> exp -> av pair.
  - Logit PSUM [128,2,512] double-buffered: exp(step n) overlaps the
    w matmuls of step n+1 (the old version single-buffered this and
    stalled the PE ~2.1us per iteration, which also kept HAM at 1.2GHz).
  - exp runs on ScalarE for 5/8 s-chunks and on VectorE for 3/8 via a
    Schraudolph bitcast exp (one tensor_scalar f32->int32), balancing
    the two engines; the av accumulator (with a ones column producing
    the softmax denominator in row 64) is evacuated PSUM->SBUF by one
    ScalarE copy so the PSUM bank frees in ~1us, and the normalize
    chain (recip, partition-broadcast, multiply) runs from SBUF,
    software-pipelined one unit behind.
  - Biases: k bias dropped (adds only t-constant terms to the logits,
    which softmax over s cancels); v bias folded into the proj bias on
    the host (p_b' = proj_b + proj_w @ v_b); q bias fused into the
    ScalarE PSUM->SBUF evacuation (activation Identity with AP bias).
  - Batch 1's groupnorm stats, hid affine, v/k/q projections are
    interleaved into batch 0's attention emission so every engine's
    FIFO order matches data-ready order and the PE never idles long
    enough to re-throttle.
"""

import os
import numpy as np

import concourse.bass as bass
import concourse.tile as tile
from concourse import mybir, bacc
from concourse.bass import ds, ts

F32 = mybir.dt.float32
I32 = mybir.dt.int32
AF = mybir.ActivationFunctionType
ALU = mybir.AluOpType

# ---- problem constants (hardcoded per contract) ----
B = 16          # global batch
BPC = 2         # batches per core
NCORES = 8
C = 512         # channels
HW = 32
L = HW * HW     # 1024 sequence positions
H = 8           # heads
CH = C // H     # 64 head dim
NG = 32         # groups
GS = C // NG    # 16 channels per group
EPS = 1e-5
CC = C // 128   # 4 channel chunks of 128
LOGIT_SCALE = 1.0 / (CH ** 0.5)  # 1/8, folded into the exp

# matmul compute dtype: f32r (1 cyc/row at N>=256, TF32-ish) default.
_DT_NAMES = {"f32": F32, "f32r": mybir.dt.float32r, "bf16": mybir.dt.bfloat16}
MM_DT = _DT_NAMES[os.environ.get("ATTN_MM_DT", "f32r")]
W_DT = MM_DT if MM_DT == mybir.dt.bfloat16 else F32

# which s-chunk steps (of 8 per unit) compute exp on VectorE via the
# Schraudolph bitcast trick instead of ScalarE. Empty string = all ACT.
_dve_env = os.environ.get("ATTN_DVE_EXP", "")
DVE_STEPS = frozenset(int(t) for t in _dve_env.split(",") if t != "")
if MM_DT == mybir.dt.bfloat16:
    DVE_STEPS = frozenset()  # bitcast trick needs 4-byte ew

# Schraudolph constants: exp(s*x) ~= bitcast_f32(int32(x*SA + SB))
LOG2E = 1.4426950408889634
SCHRAU_A = LOGIT_SCALE * LOG2E * (1 << 23)
SCHRAU_C = 0.043  # HW-measured: max rel err 3.0% with round-to-nearest
SCHRAU_B = (127.0 - SCHRAU_C) * (1 << 23)


# PSUM->SBUF evacuation engine: DVE when ScalarE is exp-bound (no DVE exp
# steps), ScalarE when the exp work is split.
EVAC = os.environ.get("ATTN_EVAC", "act" if DVE_STEPS else "dve")


def _ld(ap):
    """View a DRAM fp32 AP as the matmul dtype for direct DMA (f32r only)."""
    if MM_DT == mybir.dt.float32r:
        return ap.bitcast(MM_DT)
    return ap


def build_nc():
    nc = bacc.Bacc(name="attn_block")

    x_d = nc.dram_tensor("x", (BPC, C, L), F32, kind="ExternalInput")
    qkwt_d = nc.dram_tensor("qk_wt", (C, 2 * C), W_DT, kind="ExternalInput")
    qkb_d = nc.dram_tensor("qk_b", (2 * C,), F32, kind="ExternalInput")
    vwt_d = nc.dram_tensor("v_wt", (C, C), W_DT, kind="ExternalInput")
    pwt_d = nc.dram_tensor("p_wt", (C, C), W_DT, kind="ExternalInput")
    pb2_d = nc.dram_tensor("p_b2", (C,), F32, kind="ExternalInput")
    nw_d = nc.dram_tensor("n_w", (C,), F32, kind="ExternalInput")
    nb_d = nc.dram_tensor("n_b", (C,), F32, kind="ExternalInput")
    gmat_d = nc.dram_tensor("g_mat", (128, 8), F32, kind="ExternalInput")
    gmatt_d = nc.dram_tensor("g_mat_t", (8, 128), F32, kind="ExternalInput")
    out_d = nc.dram_tensor("out", (BPC, C, L), F32, kind="ExternalOutput")

    with tile.TileContext(nc) as tc:
        with (
            tc.tile_pool(name="wpool", bufs=1) as wpool,
            tc.tile_pool(name="big", bufs=1) as big,
            tc.tile_pool(name="work", bufs=2) as work,
            tc.tile_pool(name="small", bufs=2) as small,
            tc.tile_pool(name="psum", bufs=1, space="PSUM") as psum,
        ):
            # ---------- load inputs (x batch 0 first: groupnorm is the
            # startup critical path; weights ordered by first use) ----------
            x_tiles = []
            for b in range(BPC):
                x_sb = big.tile([128, CC, L], F32, tag="x_sb", bufs=2, name=f"x{b}")
                x_tiles.append(x_sb)
            for cc in range(CC):
                nc.sync.dma_start(
                    x_tiles[0][:, cc],
                    x_d[0].rearrange("(cc p) l -> p cc l", p=128)[:, cc],
                )
            n_w = wpool.tile([128, CC], F32)
            nc.sync.dma_start(n_w, nw_d[:].rearrange("(cc p) -> p cc", p=128))
            n_b = wpool.tile([128, CC], F32)
            nc.sync.dma_start(n_b, nb_d[:].rearrange("(cc p) -> p cc", p=128))
            g_mat = wpool.tile([128, 8], F32)
            nc.sync.dma_start(g_mat, gmat_d[:])
            g_mat_t = wpool.tile([8, 128], F32)
            nc.sync.dma_start(g_mat_t, gmatt_d[:])
            qk_b = wpool.tile([128, 8], F32)
            nc.sync.dma_start(qk_b, qkb_d[:].rearrange("(oc p) -> p oc", p=128))
            p_b2 = wpool.tile([128, CC], F32)
            nc.sync.dma_start(p_b2, pb2_d[:].rearrange("(cc p) -> p cc", p=128))
            v_wt = wpool.tile([128, CC, C], MM_DT)
            nc.sync.dma_start(v_wt, _ld(vwt_d[:]).rearrange("(cc p) o -> p cc o", p=128))
            nc.sync.dma_start(
                x_tiles[1], x_d[1].rearrange("(cc p) l -> p cc l", p=128)
            )
            qk_wt = wpool.tile([128, CC, 2 * C], MM_DT)
            nc.sync.dma_start(qk_wt, _ld(qkwt_d[:]).rearrange("(cc p) o -> p cc o", p=128))
            p_wt = wpool.tile([128, CC, C], MM_DT)
            nc.sync.dma_start(p_wt, _ld(pwt_d[:]).rearrange("(cc p) o -> p cc o", p=128))
            ones_sb = wpool.tile([128, 8, H], F32)
            nc.vector.memset(ones_sb, 1.0)

            # ---------- groupnorm stats (emitted per batch so the DVE
            # FIFO order matches data-ready order) ----------
            schs, tchs = [None, None], [None, None]

            def emit_stats(b):
                x_sb = x_tiles[b]
                stats = small.tile([128, CC, 2, 6], F32, tag="stats")
                for cc in range(CC):
                    for sub in range(2):
                        nc.vector.bn_stats(
                            out=stats[:, cc, sub],
                            in_=x_sb[:, cc, ds(sub * 512, 512)],
                        )
                mv = small.tile([128, CC, 2], F32, tag="mv")
                for cc in range(CC):
                    nc.vector.bn_aggr(out=mv[:, cc], in_=stats[:, cc])
                # mv[:, cc, 0] = mean, mv[:, cc, 1] = var -> E[x^2]
                msq = small.tile([128, CC], F32, tag="msq")
                nc.vector.tensor_tensor(msq, mv[:, :, 0], mv[:, :, 0], ALU.mult)
                nc.vector.tensor_tensor(mv[:, :, 1], mv[:, :, 1], msq, ALU.add)
                # per-group sums via PE against 0/1 group-indicator matrix
                gsum_ps = psum.tile([8, 8], F32, tag="mm", bufs=2)
                nc.tensor.matmul(
                    gsum_ps, g_mat, mv.rearrange("p a s -> p (a s)"),
                    start=True, stop=True,
                )
                gm2 = small.tile([8, CC, 2], F32, tag="gm2")
                nc.vector.tensor_scalar_mul(
                    gm2.rearrange("j a s -> j (a s)"), gsum_ps, 1.0 / GS
                )
                gsq = small.tile([8, CC], F32, tag="gsq")
                nc.vector.tensor_tensor(gsq, gm2[:, :, 0], gm2[:, :, 0], ALU.mult)
                varg = small.tile([8, CC], F32, tag="varg")
                nc.vector.tensor_tensor(varg, gm2[:, :, 1], gsq, ALU.subtract)
                nc.vector.tensor_scalar_add(varg, varg, EPS)
                # rsqrt via magic constant + 3 Newton iterations (DVE only)
                y = small.tile([8, CC], F32, tag="rsqy")
                yi = y.bitcast(I32)
                nc.vector.tensor_scalar(
                    yi, varg.bitcast(I32), 1, None, op0=ALU.logical_shift_right
                )
                nc.vector.tensor_scalar(
                    yi, yi, 0x5F3759DF, -1, op0=ALU.subtract, op1=ALU.mult
                )
                t1 = small.tile([8, CC], F32, tag="rsqt")
                for _ in range(3):
                    nc.vector.tensor_tensor(t1, y, y, ALU.mult)
                    nc.vector.tensor_tensor(t1, t1, varg, ALU.mult)
                    nc.vector.tensor_scalar(
                        t1, t1, -0.5, 1.5, op0=ALU.mult, op1=ALU.add
                    )
                    nc.vector.tensor_tensor(y, y, t1, ALU.mult)
                nc.vector.tensor_copy(out=gm2[:, :, 1], in_=y)
                # distribute group stats to channels via PE
                cstat_ps = psum.tile([128, 8], F32, tag="mm", bufs=2)
                nc.tensor.matmul(
                    cstat_ps, g_mat_t, gm2.rearrange("j a s -> j (a s)"),
                    start=True, stop=True,
                )
                cstat = cstat_ps.rearrange("p (a s) -> p a s", s=2)
                s_ch = small.tile([128, CC], F32, tag="s_ch")
                nc.vector.tensor_tensor(s_ch, cstat[:, :, 1], n_w, ALU.mult)
                t_ch = small.tile([128, CC], F32, tag="t_ch")
                nc.vector.tensor_tensor(t_ch, cstat[:, :, 0], s_ch, ALU.mult)
                nc.vector.tensor_tensor(t_ch, n_b, t_ch, ALU.subtract)
                schs[b] = s_ch
                tchs[b] = t_ch

            # ---------- per-batch persistent tiles ----------
            hid_t = [None, None]
            vt_t = [None, None]
            k_t = [None, None]
            qj_t = [[None, None], [None, None]]   # [b][j]
            aall_t = [[None, None], [None, None]]

            # ---------- emission helpers ----------
            def emit_hid(b):
                hid = big.tile([128, CC, L], MM_DT, tag="hid", bufs=1, name=f"hid{b}")
                hid_t[b] = hid
                for cc in range(CC):
                    nc.vector.tensor_scalar(
                        hid[:, cc], x_tiles[b][:, cc],
                        schs[b][:, ds(cc, 1)], tchs[b][:, ds(cc, 1)],
                        op0=ALU.mult, op1=ALU.add,
                    )

            def emit_vt_init(b):
                vt = big.tile([128, 8, H, CH + 1], MM_DT, tag="vt", bufs=2, name=f"vt{b}")
                vt_t[b] = vt
                nc.vector.tensor_copy(out=vt[:, :, :, CH], in_=ones_sb)

            def emit_v_group(b, lc):
                # out[l, o] = sum_c hid[c,l] * v_w[o,c]
                hid = hid_t[b]
                vps = psum.tile([128, C], F32, tag="mm", bufs=2)
                for cc in range(CC):
                    nc.tensor.matmul(
                        vps, hid[:, cc, ds(lc * 128, 128)], v_wt[:, cc],
                        start=(cc == 0), stop=(cc == CC - 1),
                    )
                if EVAC == "act":
                    nc.scalar.copy(
                        vt_t[b][:, lc, :, 0:CH],
                        vps.rearrange("p (h c) -> p h c", c=CH),
                    )
                else:
                    nc.vector.tensor_copy(
                        out=vt_t[b][:, lc, :, 0:CH],
                        in_=vps.rearrange("p (h c) -> p h c", c=CH),
                    )

            def emit_k_init(b):
                k_all = big.tile([128, 4, L], MM_DT, tag="k_all", bufs=2, name=f"k_all{b}")
                k_t[b] = k_all

            def emit_k_group(b, hp, lc2):
                hid = hid_t[b]
                oc = 2 * hp + 1
                kps = psum.tile([128, 512], F32, tag="mm", bufs=2)
                for cc in range(CC):
                    nc.tensor.matmul(
                        kps, qk_wt[:, cc, ds(oc * 128, 128)],
                        hid[:, cc, ds(lc2 * 512, 512)],
                        start=(cc == 0), stop=(cc == CC - 1),
                    )
                # k bias dropped: it only shifts logits by t-constants
                if EVAC == "act":
                    nc.scalar.copy(k_t[b][:, hp, ds(lc2 * 512, 512)], kps)
                else:
                    nc.vector.tensor_copy(
                        out=k_t[b][:, hp, ds(lc2 * 512, 512)], in_=kps)

            def emit_qj_init(b, j):
                qj = big.tile([128, 4, 512], MM_DT, tag="q_j", bufs=2, name=f"qj{b}_{j}")
                qj_t[b][j] = qj

            def emit_q_group(b, j, hp):
                hid = hid_t[b]
                oc = 2 * hp
                qps = psum.tile([128, 512], F32, tag="mm", bufs=2)
                for cc in range(CC):
                    nc.tensor.matmul(
                        qps, qk_wt[:, cc, ds(oc * 128, 128)],
                        hid[:, cc, ds(j * 512, 512)],
                        start=(cc == 0), stop=(cc == CC - 1),
                    )
                # q bias fused into the PSUM->SBUF evacuation
                if EVAC == "act":
                    nc.scalar.activation(
                        qj_t[b][j][:, hp], qps, AF.Identity,
                        bias=qk_b[:, ds(oc, 1)],
                    )
                else:
                    nc.vector.tensor_scalar(
                        qj_t[b][j][:, hp], qps, qk_b[:, ds(oc, 1)], None,
                        op0=ALU.add,
                    )

            def emit_proj(b, j):
                a_all = aall_t[b][j]
                for oc4 in range(CC):
                    pps = psum.tile([128, 512], F32, tag="mm", bufs=2)
                    for cc in range(CC):
                        nc.tensor.matmul(
                            pps, p_wt[:, cc, ds(oc4 * 128, 128)],
                            a_all[:, cc],
                            start=(cc == 0), stop=(cc == CC - 1),
                        )
                    o_sb = work.tile([128, 512], F32, tag="o_sb", bufs=2)
                    nc.vector.scalar_tensor_tensor(
                        o_sb, pps, p_b2[:, ds(oc4, 1)],
                        x_tiles[b][:, oc4, ds(j * 512, 512)],
                        op0=ALU.add, op1=ALU.add,
                    )
                    nc.sync.dma_start(
                        out_d[b].rearrange("(cc p) l -> p cc l", p=128)[
                            :, oc4, ds(j * 512, 512)
                        ],
                        o_sb,
                    )

            def attn_phase(b, j, interleave):
                """Flat 32-step software pipeline over (hp-unit, s-chunk).

                interleave: list of callables; consumed one per slot at
                steps i in {1, 3, 5} of each unit, leftovers at drain.
                """
                k_all, qj, vt = k_t[b], qj_t[b][j], vt_t[b]
                a_all = big.tile([128, 4, 512], MM_DT, tag="a_all", bufs=2, name=f"a_all{b}_{j}")
                aall_t[b][j] = a_all
                il = list(interleave)

                pend_av = None      # (u, i, ew)
                av2_t = {}          # u -> av psum tile
                stage = {}          # u -> dict with avun/r0/rr/rrep tiles

                def emit_w(u, i):
                    wps = psum.tile([128, 2, 512], F32, tag="wab", bufs=2)
                    for hh in range(2):
                        nc.tensor.matmul(
                            wps[:, hh],
                            k_all[ds(hh * 64, 64), u, ds(i * 128, 128)],
                            qj[ds(hh * 64, 64), u],
                            start=True, stop=True,
                            tile_position=(hh * 64, 0),
                        )
                    return wps

                def emit_exp(u, i, wps):
                    ew = work.tile([128, 2, 512], MM_DT, tag="ew", bufs=2)
                    if i in DVE_STEPS:
                        sc = work.tile([128, 2, 512], I32, tag="esc", bufs=2,
                                       name=f"esc_{b}_{j}_{u}_{i}")
                        nc.vector.tensor_scalar(
                            sc.rearrange("p a b -> p (a b)"),
                            wps.rearrange("p a b -> p (a b)"),
                            SCHRAU_A, SCHRAU_B, op0=ALU.mult, op1=ALU.add,
                        )
                        nc.sync.dma_start(ew, sc.bitcast(MM_DT))
                    else:
                        nc.scalar.activation(
                            out=ew.rearrange("p a b -> p (a b)"),
                            in_=wps.rearrange("p a b -> p (a b)"),
                            func=AF.Exp, scale=LOGIT_SCALE,
                        )
                    return ew

                def emit_av(u, i, ew):
                    if i == 0:
                        av2_t[u] = psum.tile([CH + 1, 2, 512], F32, tag="av", bufs=1, name=f"av2_{b}_{j}_{u}")
                    av2 = av2_t[u]
                    for hh in range(2):
                        nc.tensor.matmul(
                            av2[:, hh], vt[:, i, 2 * u + hh], ew[:, hh],
                            start=(i == 0), stop=(i == 7),
                        )

                def norm_a(u):  # evacuate av psum + extract denominator row
                    st = {}
                    st["avun"] = work.tile([CH + 1, 2, 512], F32, tag="avun", bufs=1, name=f"avun_{b}_{j}_{u}")
                    if EVAC == "act":
                        nc.scalar.copy(st["avun"], av2_t[u])
                    else:
                        nc.vector.tensor_copy(out=st["avun"], in_=av2_t[u])
                    st["r0"] = work.tile([1, 2, 512], F32, tag="r0", bufs=1, name=f"r0_{b}_{j}_{u}")
                    nc.sync.dma_start(st["r0"], st["avun"][ds(CH, 1)])
                    stage[u] = st

                def norm_b(u):  # reciprocal + partition-broadcast
                    st = stage[u]
                    st["rr"] = work.tile([1, 2, 512], F32, tag="rr", bufs=1, name=f"rr_{b}_{j}_{u}")
                    nc.vector.reciprocal_approx_fast(st["rr"], st["r0"])
                    st["rrep"] = work.tile([64, 2, 512], F32, tag="rrep", bufs=1, name=f"rrep_{b}_{j}_{u}")
                    nc.gpsimd.partition_broadcast(st["rrep"], st["rr"])

                def norm_c(u):  # normalize into a_all
                    st = stage.pop(u)
                    for hh in range(2):
                        nc.vector.tensor_tensor(
                            a_all[ds(hh * 64, 64), u],
                            st["avun"][0:CH, hh],
                            st["rrep"][:, hh],
                            ALU.mult,
                        )

                for u in range(4):
                    for i in range(8):
                        wps = emit_w(u, i)
                        if pend_av is not None:
                            emit_av(*pend_av)
                            pu, pi, _ = pend_av
                            if pi == 7:
                                norm_a(pu)
                            pend_av = None
                        if i == 2 and u > 0:
                            norm_b(u - 1)
                        if i == 4 and u > 0:
                            norm_c(u - 1)
                        if i in (1, 3, 5) and il:
                            il.pop(0)()
                        ew = emit_exp(u, i, wps)
                        pend_av = (u, i, ew)
                # drain
                emit_av(*pend_av)
                norm_a(3)
                norm_b(3)
                norm_c(3)
                for f in il:
                    f()
                emit_proj(b, j)

            # ---------- batch 0 prologue ----------
            emit_stats(0)
            emit_hid(0)
            emit_vt_init(0)
            for lc in range(8):
                emit_v_group(0, lc)
            emit_k_init(0)
            for hp in range(4):
                for lc2 in range(2):
                    emit_k_group(0, hp, lc2)
            emit_qj_init(0, 0)
            for hp in range(4):
                emit_q_group(0, 0, hp)
            # batch-1 stats here: their DVE ops run during batch-0's PE-dense
            # prologue, after hid(0) in the DVE FIFO
            emit_stats(1)

            # ---------- C(b0, j0) with interleaved q(b0,j1), hid(b1), v(b1) ----------
            emit_qj_init(0, 1)
            emit_vt_init(1)
            il_j0 = [lambda hp=hp: emit_q_group(0, 1, hp) for hp in range(3)]
            il_j0 += [lambda: emit_hid(1)]
            il_j0 += [lambda: emit_q_group(0, 1, 3)]
            il_j0 += [lambda lc=lc: emit_v_group(1, lc) for lc in range(8)]
            attn_phase(0, 0, il_j0)

            # ---------- C(b0, j1) with interleaved k(b1), q(b1,j0) ----------
            emit_k_init(1)
            il_j1 = [
                lambda hp=hp, lc2=lc2: emit_k_group(1, hp, lc2)
                for hp in range(4) for lc2 in range(2)
            ]
            emit_qj_init(1, 0)
            il_j1 += [lambda hp=hp: emit_q_group(1, 0, hp) for hp in range(4)]
            attn_phase(0, 1, il_j1)

            # ---------- C(b1, j0) with interleaved q(b1,j1) ----------
            emit_qj_init(1, 1)
            il_b1 = [lambda hp=hp: emit_q_group(1, 1, hp) for hp in range(4)]
            attn_phase(1, 0, il_b1)

            # ---------- C(b1, j1) ----------
            attn_phase(1, 1, [])

    nc.finalize()
    return nc


def prep_inputs(inputs):
    """Host-side weight permutation / transposition; returns per-core in_maps."""
    x = np.asarray(inputs["x"], np.float32).reshape(B, C, L)
    qkv_w = np.asarray(inputs["qkv_w"], np.float32)
    qkv_b = np.asarray(inputs["qkv_b"], np.float32)
    proj_w = np.asarray(inputs["proj_w"], np.float32)
    proj_b = np.asarray(inputs["proj_b"], np.float32)
    norm_w = np.asarray(inputs["norm_w"], np.float32)
    norm_b = np.asarray(inputs["norm_b"], np.float32)

    w3 = qkv_w.reshape(H, 3, CH, C)   # [head, (q,k,v), ch, c_in]
    b3 = qkv_b.reshape(H, 3, CH)
    # qk: per head pair -> [q_h0, q_h1] / [k_h0, k_h1] blocks of 64 rows
    qk_rows, qk_brows = [], []
    for hp in range(4):
        for which in (0, 1):
            for h in (2 * hp, 2 * hp + 1):
                qk_rows.append(w3[h, which])
                qk_brows.append(b3[h, which])
    qk_w_perm = np.concatenate(qk_rows, 0)          # [1024, 512]
    qk_wt = np.ascontiguousarray(qk_w_perm.T)       # [512, 1024]
    qk_b = np.concatenate(qk_brows, 0)              # [1024]
    v_w_perm = w3[:, 2].reshape(C, C)               # head-major v rows
    v_wt = np.ascontiguousarray(v_w_perm.T)         # [512, 512]
    v_b = b3[:, 2].reshape(C)
    p_wt = np.ascontiguousarray(proj_w.T)
    # v bias folded into the proj bias: proj(a + bv) = proj(a) + P@bv
    p_b2 = proj_b + proj_w @ v_b
    g_mat = np.zeros((128, 8), np.float32)
    g_mat[np.arange(128), np.arange(128) // 16] = 1.0
    g_mat_t = np.ascontiguousarray(g_mat.T)

    if MM_DT == mybir.dt.bfloat16:
        import ml_dtypes
        bf = ml_dtypes.bfloat16
        qk_wt = qk_wt.astype(bf)
        v_wt = v_wt.astype(bf)
        p_wt = p_wt.astype(bf)
    shared = {
        "qk_wt": qk_wt, "qk_b": qk_b, "v_wt": v_wt,
        "p_wt": p_wt, "p_b2": p_b2.astype(np.float32),
        "n_w": norm_w, "n_b": norm_b,
        "g_mat": g_mat, "g_mat_t": g_mat_t,
    }
    in_maps = []
    for c in range(NCORES):
        m = dict(shared)
        m["x"] = np.ascontiguousarray(x[c * BPC: (c + 1) * BPC])
        in_maps.append(m)
    return in_maps


_NC_CACHE = {}


def get_nc():
    key = (str(MM_DT), tuple(sorted(DVE_STEPS)))
    if key not in _NC_CACHE:
        _NC_CACHE[key] = build_nc()
    return _NC_CACHE[key]


def kernel(**inputs) -> np.ndarray:
    from concourse import bass_utils

    nc = get_nc()
    in_maps = prep_inputs(inputs)
    res = bass_utils.run_bass_kernel_spmd(nc, in_maps, core_ids=list(range(NCORES)))
    outs = [res.results[c]["out"] for c in range(NCORES)]
    full = np.concatenate(outs, 0).reshape(B, C, HW, HW)
    return full.astype(np.float32)
